# revision 6
# baseline (speedup 1.0000x reference)
"""Trainium2 Bass kernel v6 for windowed multi-head attention (2.5D swin).

Problem (hardcoded from spec nn_Attention25d_86775519248925):
  x:          (4, 16, 16, 8, 7, 7, 1, 128) f32  -> B=8192 windows, n=49 tokens, d=128
  w_qkv:      (128, 384) f32
  w_out:      (128, 128) f32
  bias_table: (169, 4) f32
  out:        same shape as x

Sharding: pure data parallel over the fused window-batch axis across 8 cores.

v6 design (v5 + j-axis padding for 2-way row-group concurrency on the
attn@v / Z matmuls, which were the largest serial tensor cost):
  - j (key) slots padded 98 -> 113 per pair: window0 keys at partitions
    0..48, zeros at 49..63, window1 keys at 64..112.  Host pads xT.
  - attn@v and Z split per window: w0 contracts K=64 (rows 0..63),
    w1 contracts K=49 at base partition 64 (rows 64..127).  The two
    window streams use disjoint PE row groups -> concurrent, writing two
    different psum banks (zyP for w0, zyQ for w1; each holds y and Z).
  - cross-window + pad masking costs NO extra aug rank: the two rank-1
    mask rows get sqrt(L) also at pad columns, so pad rows see -L for
    every query -> exp ~ 1e-11.
  - i (query) columns stay compact 98: q / sim-rhs / attn / y / fin /
    out are all 98-wide as in v5.

From v5: single scalar qkT copy; exp batched 2 heads per ACTIVATE; sim
4-head round-robin; bf16 out [g, 98, 4, 128]; {v, fin, zyP, zyQ} 2-bank
psum ring; SVD-folded bias (rank 30 + 2 mask rows per head).

Hardware constraints honored (probed):
  - concurrent tile-position matmuls from different row-groups must write
    different PSUM banks.
  - no PSUM accumulation chains across row-groups (the w0/w1 split writes
    DISJOINT outputs instead: masked cross-window attn contributes ~0).
  - DMA cannot touch PSUM.
"""

import os
import sys
import threading

import numpy as np

for _p in ("/opt/trn_rl_repo", "/root/.axon_site/_ro/trn_rl_repo"):
    if os.path.isdir(_p) and _p not in sys.path:
        sys.path.insert(0, _p)

# ---------------------------------------------------------------- constants
WS = 7
N_TOK = 49            # tokens per window
D = 128
H = 4
DH = 32
SCALE = DH ** -0.5
B_FULL = 4 * 16 * 16 * 8   # 8192 windows
N_CORES = 8
NI = 2 * N_TOK        # 98 compact query/i slots per pair
NJ = 113              # padded key/j slots per pair: 49 + 15 pad + 49


def _rel_pos_bias(bias_table: np.ndarray) -> np.ndarray:
    """bias[h, i, j] from the 169x4 table (numpy copy of reference logic)."""
    pos = np.arange(WS)
    gi, gj = np.meshgrid(pos, pos, indexing="ij")
    grid = np.stack([gi.reshape(-1), gj.reshape(-1)], axis=-1)
    rel = grid[:, None, :] - grid[None, :, :] + (WS - 1)
    idx = rel[..., 0] * (2 * WS - 1) + rel[..., 1]            # (49, 49)
    b = bias_table[idx]                                       # (49, 49, 4)
    return np.transpose(b, (2, 0, 1)).astype(np.float32)      # (h, i, j)


def _host_constants(w_qkv, w_out, bias_table):
    import ml_dtypes
    bf = ml_dtypes.bfloat16
    wq = np.ascontiguousarray((w_qkv[:, :D] * SCALE).astype(bf))
    wk = np.ascontiguousarray(w_qkv[:, D:2 * D].astype(bf))
    wv = np.ascontiguousarray(w_qkv[:, 2 * D:].astype(bf))
    wo = np.ascontiguousarray(w_out.astype(bf))

    bias = _rel_pos_bias(np.asarray(bias_table, dtype=np.float32))  # (h,i,j)
    # Fold bias + masks into the sim matmul contraction. Per head, 32
    # augmented rows: rank-30 SVD of the 49x49 bias block + 2 rank-1 mask
    # rows. augK spans the padded j axis (113), augQ the compact i axis
    # (98). Mask rows subtract L from cross-window logits AND (via sqrt(L)
    # at the pad columns of BOTH rows) from every pad-row logit.
    L = 25.0
    sL = np.float32(np.sqrt(L))
    augK = np.zeros((128, NJ), dtype=np.float32)
    augQ = np.zeros((128, NI), dtype=np.float32)
    for h in range(H):
        Bp = bias[h].T                                    # B'[j_tok, i_tok]
        U, s, Vt = np.linalg.svd(Bp)
        r = 30
        P = U[:, :r] * np.sqrt(s[:r])                     # (49, r)
        Q = np.sqrt(s[:r])[:, None] * Vt[:r, :]           # (r, 49)
        augK[DH * h: DH * h + r, 0:N_TOK] = P.T
        augK[DH * h: DH * h + r, 64:64 + N_TOK] = P.T
        augQ[DH * h: DH * h + r, :] = np.tile(Q, (1, 2))
        # mask row 30: -L for (j in w0 or pad) x (i in w1)
        augK[DH * h + 30, 0:64] = sL
        augQ[DH * h + 30, N_TOK:] = -sL
        # mask row 31: -L for (j in w1 or pad) x (i in w0)
        augK[DH * h + 31, 49:NJ] = sL
        augQ[DH * h + 31, :N_TOK] = -sL
    augK = augK.astype(bf)
    augQ = augQ.astype(bf)
    ones32 = np.ones((NJ, DH), dtype=bf)
    return dict(wq=wq, wk=wk, wv=wv, wo=wo, augK=augK, augQ=augQ,
                ones32=ones32)


def _host_xT(x_tokens: np.ndarray, n_windows: int) -> np.ndarray:
    """xT [128(d), n_groups, 4(pair), 113(j-slot)] bf16, j-padded, from
    token-major x [nt, 128] f32 for one core."""
    import ml_dtypes
    bf = ml_dtypes.bfloat16
    n_groups = n_windows // 8
    xw = x_tokens.reshape(n_groups, 4, 2, N_TOK, D)
    xT = np.zeros((D, n_groups, 4, NJ), dtype=bf)
    xT[:, :, :, 0:N_TOK] = xw[:, :, 0].transpose(3, 0, 1, 2)
    xT[:, :, :, 64:64 + N_TOK] = xw[:, :, 1].transpose(3, 0, 1, 2)
    return np.ascontiguousarray(xT)


def _build_bass(n_windows: int):
    """Build the Bass/Tile program for one core processing n_windows windows."""
    import concourse.bacc as bacc
    import concourse.bass as bass
    import concourse.mybir as mybir
    import concourse.tile as tile

    f32 = mybir.dt.float32
    bf = mybir.dt.bfloat16
    n_groups = n_windows // 8
    assert n_windows % 8 == 0

    nc = bacc.Bacc("TRN2", target_bir_lowering=False, debug=False,
                   enable_asserts=False)

    xT_d = nc.dram_tensor("xT", [D, n_groups, 4, NJ], bf, kind="ExternalInput")
    out_t = nc.dram_tensor("out", [n_groups, NI, 4, D], bf,
                           kind="ExternalOutput")
    wq_d = nc.dram_tensor("wq", [D, D], bf, kind="ExternalInput")
    wk_d = nc.dram_tensor("wk", [D, D], bf, kind="ExternalInput")
    wv_d = nc.dram_tensor("wv", [D, D], bf, kind="ExternalInput")
    wo_d = nc.dram_tensor("wo", [D, D], bf, kind="ExternalInput")
    augk_d = nc.dram_tensor("augK", [128, NJ], bf, kind="ExternalInput")
    augq_d = nc.dram_tensor("augQ", [128, NI], bf, kind="ExternalInput")
    ones_d = nc.dram_tensor("ones32", [NJ, DH], bf, kind="ExternalInput")

    with tile.TileContext(nc) as tc:
        with (
            tc.tile_pool(name="singles", bufs=1) as singles,
            tc.tile_pool(name="xt", bufs=4) as pool_xt,
            tc.tile_pool(name="qk", bufs=2) as pool_qk,
            tc.tile_pool(name="vsb", bufs=3) as pool_v,
            tc.tile_pool(name="attn", bufs=4) as pool_attn,
            tc.tile_pool(name="rz", bufs=4) as pool_rz,
            tc.tile_pool(name="ysb", bufs=3) as pool_y,
            tc.tile_pool(name="outb", bufs=3) as pool_out,
            tc.tile_pool(name="psS", bufs=2, space="PSUM") as pool_sim,
            tc.tile_pool(name="psQK", bufs=1, space="PSUM") as pool_pqk,
            tc.tile_pool(name="psV", bufs=2, space="PSUM") as pool_vfy,
        ):
            wq_sb = singles.tile([D, D], bf, tag="wq")
            wk_sb = singles.tile([D, D], bf, tag="wk")
            wv_sb = singles.tile([D, D], bf, tag="wv")
            wo_sb = singles.tile([D, D], bf, tag="wo")
            augk_sb = singles.tile([128, NJ], bf, tag="augK")
            augq_sb = singles.tile([128, NI], bf, tag="augQ")
            ones_sb = singles.tile([NJ, DH], bf, tag="ones")
            for sb, dr in ((wq_sb, wq_d), (wk_sb, wk_d), (wv_sb, wv_d),
                           (wo_sb, wo_d), (augk_sb, augk_d),
                           (augq_sb, augq_d), (ones_sb, ones_d)):
                nc.sync.dma_start(out=sb[:], in_=dr[:])

            def emit_fin(y_sb, g):
                ps_f = pool_vfy.tile([128, 4, 128], f32, tag="v")
                for p in range(4):
                    nc.tensor.matmul(ps_f[:NI, p, :], y_sb[:, p, :], wo_sb[:])
                outb = pool_out.tile([NI, 4, D], bf, tag="outb")
                nc.vector.tensor_copy(outb[:], ps_f[:NI, :, :])
                nc.sync.dma_start(out=out_t[g], in_=outb[:])

            pending = None
            for g in range(n_groups):
                # ---- input: straight DMA of host-padded xT ---------------
                xT = pool_xt.tile([128, 4, NJ], bf, tag="xt")
                nc.sync.dma_start(out=xT[:], in_=xT_d[:, g, :, :])

                # ---- q (i-compact 98, two window halves) and k (113) -----
                ps_qk = pool_pqk.tile([128, 2, 4, 128], f32, tag="qk")
                nc.tensor.matmul(ps_qk[:, 0, :, 0:N_TOK], wq_sb[:],
                                 xT[:, :, 0:N_TOK])
                nc.tensor.matmul(ps_qk[:, 0, :, N_TOK:NI], wq_sb[:],
                                 xT[:, :, 64:NJ])
                nc.tensor.matmul(ps_qk[:, 1, :, :NJ], wk_sb[:], xT[:])

                # ---- v token-major, padded rows (x pad = 0 -> v pad = 0) -
                ps_v = pool_vfy.tile([128, 4, 128], f32, tag="v")
                for p in range(4):
                    nc.tensor.matmul(ps_v[:NJ, p, :], xT[:, p, :], wv_sb[:])

                qkT = pool_qk.tile([128, 2, 4, NJ], bf, tag="qkT")
                nc.scalar.copy(qkT[:], ps_qk[:, :, :, :NJ])
                v_sb = pool_v.tile([NJ, 4, 128], bf, tag="v")
                nc.vector.tensor_copy(v_sb[:], ps_v[:NJ, :, :])

                # ---- sim^T [113 j, 98 i]: 4 heads round-robin ------------
                # psum tiles stay [.., 128]-strided so every matmul dst
                # region lives inside a single 2KB psum bank.
                psA = pool_sim.tile([128, 2, 4, 128], f32, tag="s")
                psB = pool_sim.tile([128, 2, 4, 128], f32, tag="s")
                for p in range(4):
                    for h in range(H):
                        ps_sim = psA if h < 2 else psB
                        hi = h % 2
                        nc.tensor.matmul(
                            ps_sim[:NJ, hi, p, :NI],
                            qkT[DH * h: DH * (h + 1), 1, p, :],
                            qkT[DH * h: DH * (h + 1), 0, p, :NI],
                            tile_position=(DH * h, 0), start=True,
                            stop=False)
                        nc.tensor.matmul(
                            ps_sim[:NJ, hi, p, :NI],
                            augk_sb[DH * h: DH * (h + 1), :],
                            augq_sb[DH * h: DH * (h + 1), :],
                            tile_position=(DH * h, 0), start=False,
                            stop=True)

                # ---- deferred final projection of the previous group -----
                if pending is not None:
                    emit_fin(*pending)
                    pending = None

                # ---- exp: one ACTIVATE per 2-head sim tile ---------------
                attnA = pool_attn.tile([NJ, 2, 4, NI], bf, tag="attn")
                nc.scalar.activation(attnA[:], psA[:NJ, :, :, :NI],
                                     mybir.ActivationFunctionType.Exp)
                attnB = pool_attn.tile([NJ, 2, 4, NI], bf, tag="attn")
                nc.scalar.activation(attnB[:], psB[:NJ, :, :, :NI],
                                     mybir.ActivationFunctionType.Exp)

                # ---- Z + attn@v, split by key window (2-way row groups) --
                # zyP (w0: rows 0..63)  : [:, 0] = y_w0, [:, 1] = Z_w0
                # zyQ (w1: rows 64..112): [:, 0] = y_w1, [:, 1] = Z_w1
                zyP = pool_vfy.tile([128, 2, 4, N_TOK], f32, tag="v")
                zyQ = pool_vfy.tile([128, 2, 4, N_TOK], f32, tag="v")
                for h in range(H):
                    attn_t = attnA if h < 2 else attnB
                    hi = h % 2
                    nc.tensor.matmul(
                        zyP[DH * h: DH * (h + 1), 1, :, :],
                        ones_sb[0:64, :], attn_t[0:64, hi, :, 0:N_TOK],
                        tile_position=(0, DH * h), start=True, stop=True)
                    nc.tensor.matmul(
                        zyQ[DH * h: DH * (h + 1), 1, :, :],
                        ones_sb[64:NJ, :], attn_t[64:NJ, hi, :, N_TOK:],
                        tile_position=(64, DH * h), start=True, stop=True)
                    for p in range(4):
                        nc.tensor.matmul(
                            zyP[DH * h: DH * (h + 1), 0, p, :],
                            v_sb[0:64, p, DH * h: DH * (h + 1)],
                            attn_t[0:64, hi, p, 0:N_TOK],
                            tile_position=(0, DH * h), start=True, stop=True)
                        nc.tensor.matmul(
                            zyQ[DH * h: DH * (h + 1), 0, p, :],
                            v_sb[64:NJ, p, DH * h: DH * (h + 1)],
                            attn_t[64:NJ, hi, p, N_TOK:],
                            tile_position=(64, DH * h), start=True, stop=True)

                rzP = pool_rz.tile([128, 4, N_TOK], f32, tag="rz")
                nc.vector.reciprocal_approx_fast(rzP[:], zyP[:, 1, :, :])
                y_sb = pool_y.tile([128, 4, NI], bf, tag="y")
                nc.vector.tensor_mul(y_sb[:, :, 0:N_TOK], zyP[:, 0, :, :],
                                     rzP[:])
                rzQ = pool_rz.tile([128, 4, N_TOK], f32, tag="rz")
                nc.vector.reciprocal_approx_fast(rzQ[:], zyQ[:, 1, :, :])
                nc.vector.tensor_mul(y_sb[:, :, N_TOK:], zyQ[:, 0, :, :],
                                     rzQ[:])
                pending = (y_sb, g)

            emit_fin(*pending)

    nc.compile()
    return nc


# ------------------------------------------------------------- run helpers
_CACHE = {}
_LOCK = threading.Lock()
LAST_RESULT = None


def _get_nc(n_windows: int):
    with _LOCK:
        if n_windows not in _CACHE:
            _CACHE[n_windows] = _build_bass(n_windows)
        return _CACHE[n_windows]


def kernel(x, w_qkv, w_out, bias_table):
    from concourse.bass_utils import run_bass_kernel_spmd

    global LAST_RESULT
    x = np.asarray(x, dtype=np.float32)
    b, X, Y, Z, w1, w2, w3, d = x.shape
    B = b * X * Y * Z
    assert B == B_FULL and w1 * w2 * w3 == N_TOK and d == D
    w_core = B // N_CORES
    nt = w_core * N_TOK

    consts = _host_constants(np.asarray(w_qkv, np.float32),
                             np.asarray(w_out, np.float32),
                             np.asarray(bias_table, np.float32))
    nc = _get_nc(w_core)

    xf = np.ascontiguousarray(x.reshape(B * N_TOK, D))
    in_maps = []
    for c in range(N_CORES):
        m = {"xT": _host_xT(xf[c * nt: (c + 1) * nt], w_core),
             "wq": consts["wq"], "wk": consts["wk"], "wv": consts["wv"],
             "wo": consts["wo"], "augK": consts["augK"],
             "augQ": consts["augQ"], "ones32": consts["ones32"]}
        in_maps.append(m)

    res = run_bass_kernel_spmd(nc, in_maps, core_ids=list(range(N_CORES)))
    LAST_RESULT = res
    # out bf16 [n_groups, 98(i), 4(p), 128] -> f32 token-major
    out = np.concatenate(
        [np.asarray(r["out"], dtype=np.float32)
         .transpose(0, 2, 1, 3).reshape(nt, D)
         for r in res.results], axis=0)
    return out.reshape(x.shape)
